# revision 5
# baseline (speedup 1.0000x reference)
"""CommutatorConv2d kernel for Trainium2 (Bass/Tile), 8-core data-parallel.

Math: the reference's commutator/anticommutator conv reduces exactly to a
single-channel 3x3 conv on the channel-summed input xs = x.sum(axis=1).
Writing the conv's horizontal taps as shifted copies and folding them into
the vertical band matrices gives a SINGLE matmul stage:

    out[b] = V0 @ shiftR(xs) + V1 @ xs + V2 @ shiftL(xs) + bias
    V_k = a[k]*T + Bm   (T tridiagonal-ones, Bm tridiagonal from K row
                         sums, a[k] from K column sums)

and since the V stage is linear, xs can stay SPLIT into partial sums that
each get their own shifted V matmuls into one accumulating PSUM group --
no final merge sits on the critical path.

v7 schedule (bf16 in/out, f32 accumulate in PSUM):
- sync queue:   [cmat | b1 ch0-15] fused head, then b0d (8ch), b0p3 (8ch)
  scalar queue: b1 ch16-31, b0p1 (8ch), b0p2 (8ch); batch 1 first on both
  queues so its whole tail hides under batch 0's streaming.
- b1 reduction: two 16ch DVE trees, merged by gpsimd into a zero-padded
  buffer -> 3 V matmuls.
- b0 reduction: 24ch PE identity-folds (fp32 PSUM) + one 8ch DVE tree.
  The tree lands in its own padded buffer; one tensor_reduce collapses
  the PSUM partials into another. 6 V matmuls (3 per buffer) accumulate
  both into o_psum while the reduce overlaps b1's V matmuls on PE.
- Output stored as bf16 (halves store flight), host upcasts to f32.
"""

import numpy as np

B, C, H, W = 16, 32, 128, 128
N_CORES = 8
B_LOC = B // N_CORES

CMCOLS = 4 * W + 2
N_JUNK = 8

_PROGRAM = None
LAST_RESULTS = None


def _build_program():
    import concourse.mybir as mybir
    from concourse import bacc
    from concourse.bass import MemorySpace
    from concourse.tile import TileContext

    bf16 = mybir.dt.bfloat16
    f32 = mybir.dt.float32
    nc = bacc.Bacc(
        "TRN2", target_bir_lowering=False, debug=False, num_devices=N_CORES
    )

    ncols = CMCOLS + 2 * C * W
    xc_dram = nc.dram_tensor("xc", (H, ncols), bf16, kind="ExternalInput")
    out_dram = nc.dram_tensor("out", (B_LOC, H, W), bf16, kind="ExternalOutput")

    xc_ap = xc_dram.ap()
    out_ap = out_dram.ap()

    # column layout: [cmat|b1A(16ch) | b0d(8) | b0p3(8) || b1B(16) | b0p1(8) | b0p2(8)]
    HEADC = CMCOLS + 16 * W
    c_b0d = HEADC
    c_b0p3 = c_b0d + 8 * W
    c_b1B = c_b0p3 + 8 * W
    c_b0p1 = c_b1B + 16 * W
    c_b0p2 = c_b0p1 + 8 * W

    with TileContext(nc) as tc:
        with (
            tc.tile_pool(name="xpool", bufs=1) as xpool,
            tc.tile_pool(name="spool", bufs=1) as spool,
            tc.tile_pool(name="psum", bufs=1, space=MemorySpace.PSUM) as ppool,
        ):
            # PE warmup scratch + zero-edged pad buffers (gpsimd, off-path)
            scratch = spool.tile([H, 5 * W], bf16, tag="scratch")
            nc.gpsimd.memset(scratch, 0.0)
            xsp1 = spool.tile([H, W + 2], bf16, tag="xsp1")   # b1 tree merge
            nc.gpsimd.memset(xsp1, 0.0)
            xsp0t = spool.tile([H, W + 2], bf16, tag="xsp0t")  # b0 tree
            nc.gpsimd.memset(xsp0t, 0.0)
            xsp0r = spool.tile([H, W + 2], bf16, tag="xsp0r")  # b0 reduce
            nc.gpsimd.memset(xsp0r, 0.0)

            # ---- input DMAs (b1 first on both queues) ----
            head = xpool.tile([H, HEADC], bf16, tag="head")
            nc.sync.dma_start(out=head, in_=xc_ap[:, 0:HEADC])
            cm_sb = head[:, 0:CMCOLS]
            i_sb = cm_sb[:, 3 * W : 4 * W]
            bias_sb = cm_sb[:, 4 * W : 4 * W + 2].bitcast(f32)
            b1A = head[:, CMCOLS:HEADC]

            b0d = xpool.tile([H, 8 * W], bf16, tag="b0d")
            nc.sync.dma_start(out=b0d, in_=xc_ap[:, c_b0d : c_b0d + 8 * W])
            b0p3 = xpool.tile([H, 8 * W], bf16, tag="b0p3")
            nc.sync.dma_start(out=b0p3, in_=xc_ap[:, c_b0p3 : c_b0p3 + 8 * W])

            b1B = xpool.tile([H, 16 * W], bf16, tag="b1B")
            nc.scalar.dma_start(out=b1B, in_=xc_ap[:, c_b1B : c_b1B + 16 * W])
            b0p1 = xpool.tile([H, 8 * W], bf16, tag="b0p1")
            nc.scalar.dma_start(out=b0p1, in_=xc_ap[:, c_b0p1 : c_b0p1 + 8 * W])
            b0p2 = xpool.tile([H, 8 * W], bf16, tag="b0p2")
            nc.scalar.dma_start(out=b0p2, in_=xc_ap[:, c_b0p2 : c_b0p2 + 8 * W])

            # ---- PE warmup ----
            junk_psum = ppool.tile([H, 4 * W], f32, tag="junk")
            for _ in range(N_JUNK):
                nc.tensor.matmul(
                    junk_psum,
                    scratch[:, 0:W],
                    scratch[:, W : 5 * W],
                    start=True,
                    stop=True,
                    skip_group_check=True,
                )

            fold_psum = ppool.tile([H, 4 * W], f32, tag="fold_psum")
            o_psum = {
                1: ppool.tile([H, W], f32, name="op1", tag="op1"),
                0: ppool.tile([H, W], f32, name="op0", tag="op0"),
            }

            # ---- b0: PE identity-folds of p1, p2, p3 (one PSUM group) ----
            for pi, p in enumerate((b0p1, b0p2, b0p3)):
                for c in range(2):
                    nc.tensor.matmul(
                        fold_psum,
                        i_sb,
                        p[:, c * 4 * W : (c + 1) * 4 * W],
                        start=(pi == 0 and c == 0),
                        stop=(pi == 2 and c == 1),
                        skip_group_check=True,
                    )

            # ---- b1: two 16ch DVE trees -> gpsimd merge into xsp1 ----
            def tree16(p):
                # [128, 2048] -> result at p[:, 0:W], in place
                n = 16 * W
                while n > W:
                    n //= 2
                    nc.vector.tensor_add(p[:, :n], p[:, :n], p[:, n : 2 * n])

            tree16(b1B)
            tree16(b1A)
            nc.gpsimd.tensor_add(xsp1[:, 1 : W + 1], b1A[:, 0:W], b1B[:, 0:W])

            # ---- b0: 8ch DVE tree into xsp0t; reduce PSUM into xsp0r ----
            n = 8 * W
            while n > 2 * W:
                n //= 2
                nc.vector.tensor_add(b0d[:, :n], b0d[:, :n], b0d[:, n : 2 * n])
            nc.vector.tensor_add(
                xsp0t[:, 1 : W + 1], b0d[:, 0:W], b0d[:, W : 2 * W]
            )
            with nc.allow_low_precision("bf16 partials; gate is 2e-2"):
                nc.vector.tensor_reduce(
                    xsp0r[:, 1 : W + 1],
                    fold_psum[:, 0 : 4 * W].rearrange("p (j w) -> p w j", j=4),
                    axis=mybir.AxisListType.X,
                    op=mybir.AluOpType.add,
                )

            # ---- V matmuls ----
            def vmms(psum, xsp, start, stop):
                for k in range(3):
                    nc.tensor.matmul(
                        psum,
                        cm_sb[:, k * W : (k + 1) * W],
                        xsp[:, k : k + W],
                        start=(start and k == 0),
                        stop=(stop and k == 2),
                        skip_group_check=True,
                    )

            vmms(o_psum[1], xsp1, True, True)
            vmms(o_psum[0], xsp0t, True, False)
            vmms(o_psum[0], xsp0r, False, True)

            # ---- bias evac (bf16 out) + stores split across both queues ----
            for b in (1, 0):
                osb = spool.tile([H, W], bf16, name=f"o{b}", tag=f"o{b}")
                with nc.allow_low_precision("bf16 output; gate is 2e-2"):
                    nc.scalar.add(osb, o_psum[b], add=bias_sb)
                nc.sync.dma_start(
                    out=out_ap[b, 0 : H // 2, :], in_=osb[0 : H // 2, :]
                )
                nc.scalar.dma_start(
                    out=out_ap[b, H // 2 : H, :], in_=osb[H // 2 : H, :]
                )

    nc.compile()
    return nc


def _get_program():
    global _PROGRAM
    if _PROGRAM is None:
        _PROGRAM = _build_program()
    return _PROGRAM


def _build_consts(K, bias, lambda_c, lambda_a):
    import ml_dtypes

    K = np.asarray(K, np.float32)
    lc = float(np.asarray(lambda_c))
    la = float(np.asarray(lambda_a))
    a = (lc + la) * K.sum(axis=0)  # column sums -> horizontal taps
    b = (la - lc) * K.sum(axis=1)  # row sums -> vertical taps
    eye = np.eye(H, dtype=np.float32)
    up = np.eye(H, k=1, dtype=np.float32)
    dn = np.eye(H, k=-1, dtype=np.float32)
    T = eye + up + dn
    Bm = b[1] * eye + b[2] * up + b[0] * dn
    vs = [np.ascontiguousarray((a[k] * T + Bm).T) for k in range(3)]
    cm = np.concatenate(vs + [eye], axis=1)
    cm16 = cm.astype(ml_dtypes.bfloat16)
    bias_col = np.full(
        (H, 1), np.asarray(bias, np.float32).reshape(-1)[0], np.float32
    )
    bias_bits = bias_col.view(np.uint16).view(ml_dtypes.bfloat16)  # [H, 2]
    return np.concatenate([cm16, bias_bits], axis=1)


def kernel(x, K, bias, lambda_c, lambda_a, _trace=False):
    global LAST_RESULTS
    import ml_dtypes
    from concourse.bass_utils import run_bass_kernel_spmd

    x = np.asarray(x, np.float32)
    cmb = _build_consts(K, bias, lambda_c, lambda_a)
    nc = _get_program()

    in_maps = []
    for core in range(N_CORES):
        shard = x[core * B_LOC : (core + 1) * B_LOC]  # [2, C, H, W]
        st = shard.transpose(2, 0, 1, 3).astype(ml_dtypes.bfloat16)  # [H,2,C,W]
        blocks = [
            cmb,
            st[:, 1, 0:16].reshape(H, 16 * W),    # b1A (in head)
            st[:, 0, 24:32].reshape(H, 8 * W),    # b0d
            st[:, 0, 16:24].reshape(H, 8 * W),    # b0p3
            st[:, 1, 16:32].reshape(H, 16 * W),   # b1B
            st[:, 0, 0:8].reshape(H, 8 * W),      # b0p1
            st[:, 0, 8:16].reshape(H, 8 * W),     # b0p2
        ]
        xc = np.concatenate(blocks, axis=1)
        in_maps.append({"xc": np.ascontiguousarray(xc)})

    res = run_bass_kernel_spmd(
        nc, in_maps, core_ids=list(range(N_CORES)), trace=_trace
    )
    LAST_RESULTS = res
    out = np.concatenate([np.asarray(r["out"]) for r in res.results], axis=0)
    return out.reshape(B, 1, H, W).astype(np.float32)


# revision 7
# speedup vs baseline: 1.0927x; 1.0927x over previous
"""CommutatorConv2d kernel for Trainium2 (Bass/Tile), 8-core data-parallel.

Math: the reference's commutator/anticommutator conv reduces exactly to a
single-channel 3x3 conv on the channel-summed input xs = x.sum(axis=1).
Writing the conv's horizontal taps as shifted copies and folding them into
the vertical band matrices gives a SINGLE matmul stage:

    out[b] = V0 @ shiftR(xs) + V1 @ xs + V2 @ shiftL(xs) + bias
    V_k = a[k]*T + Bm   (T tridiagonal-ones, Bm tridiagonal from K row
                         sums, a[k] from K column sums)

and since the V stage is linear, xs stays SPLIT into partial sums that
each get their own shifted V matmuls into one accumulating PSUM group --
no merge chain sits on the critical path.

v8 (bf16 in, f32 out):
- NO scalar-engine compute at all: the auto-inserted ACT_TABLE_LOAD was
  delaying the scalar queue's first DMA by ~2us.  Bias rides on
  DVE/gpsimd tensor_scalar_add instead.
- 8ch (2KB-row) pieces everywhere: larger pieces showed multi-us
  straggler tails on the last descriptors.
- b1 = 4 DVE trees merged by gpsimd; b0 = 24ch PE folds in TWO psum
  groups (p1+p3 / p2) so the first reduce overlaps the last piece's
  folds, + one 8ch DVE tree.  V matmuls per partial buffer.
"""

import numpy as np

B, C, H, W = 16, 32, 128, 128
N_CORES = 8
B_LOC = B // N_CORES

CMCOLS = 4 * W + 2
PC = 8 * W  # piece cols
N_JUNK = 8

_PROGRAM = None
LAST_RESULTS = None


def _build_program():
    import concourse.mybir as mybir
    from concourse import bacc
    from concourse.bass import MemorySpace
    from concourse.tile import TileContext

    bf16 = mybir.dt.bfloat16
    f32 = mybir.dt.float32
    nc = bacc.Bacc(
        "TRN2", target_bir_lowering=False, debug=False, num_devices=N_CORES
    )

    ncols = CMCOLS + 2 * C * W
    xc_dram = nc.dram_tensor("xc", (H, ncols), bf16, kind="ExternalInput")
    out_dram = nc.dram_tensor("out", (B_LOC, H, W), f32, kind="ExternalOutput")

    xc_ap = xc_dram.ap()
    out_ap = out_dram.ap()

    # cols: [cmat|b1d1 | b1d3 | b0d | b0p2 || b1d2 | b1d4 | b0p1 | b0p3]
    HEADC = CMCOLS + PC
    cof = {}
    c = HEADC
    for nm in ("b1d3", "b0d", "b0p2", "b1d2", "b1d4", "b0p1", "b0p3"):
        cof[nm] = c
        c += PC

    with TileContext(nc) as tc:
        with (
            tc.tile_pool(name="xpool", bufs=1) as xpool,
            tc.tile_pool(name="spool", bufs=1) as spool,
            tc.tile_pool(name="psum", bufs=1, space=MemorySpace.PSUM) as ppool,
        ):
            # PE warmup scratch + zero-edged pad buffers (gpsimd, off-path)
            scratch = spool.tile([H, 5 * W], bf16, tag="scratch")
            nc.gpsimd.memset(scratch, 0.0)
            pads = {}
            for nm in ("xsp1", "xsp0t", "xsp0ra", "xsp0rb"):
                t = spool.tile([H, W + 2], bf16, name=nm, tag=nm)
                nc.gpsimd.memset(t, 0.0)
                pads[nm] = t

            # ---- input DMAs (b1 first on both queues) ----
            head = xpool.tile([H, HEADC], bf16, tag="head")
            nc.sync.dma_start(out=head, in_=xc_ap[:, 0:HEADC])
            cm_sb = head[:, 0:CMCOLS]
            i_sb = cm_sb[:, 3 * W : 4 * W]
            bias_sb = cm_sb[:, 4 * W : 4 * W + 2].bitcast(f32)
            b1d1 = head[:, CMCOLS:HEADC]

            tiles = {}
            for nm in ("b1d3", "b0d", "b0p2"):  # sync queue
                t = xpool.tile([H, PC], bf16, name=nm, tag=nm)
                nc.sync.dma_start(out=t, in_=xc_ap[:, cof[nm] : cof[nm] + PC])
                tiles[nm] = t
            for nm in ("b1d2", "b1d4", "b0p1", "b0p3"):  # scalar queue
                t = xpool.tile([H, PC], bf16, name=nm, tag=nm)
                nc.scalar.dma_start(out=t, in_=xc_ap[:, cof[nm] : cof[nm] + PC])
                tiles[nm] = t

            # ---- PE warmup ----
            junk_psum = ppool.tile([H, 4 * W], f32, tag="junk")
            for _ in range(N_JUNK):
                nc.tensor.matmul(
                    junk_psum,
                    scratch[:, 0:W],
                    scratch[:, W : 5 * W],
                    start=True,
                    stop=True,
                    skip_group_check=True,
                )

            psumA = ppool.tile([H, 4 * W], f32, tag="psumA")
            psumB = ppool.tile([H, 4 * W], f32, tag="psumB")
            o_psum = {
                1: ppool.tile([H, W], f32, name="op1", tag="op1"),
                0: ppool.tile([H, W], f32, name="op0", tag="op0"),
            }

            def fold(psum, p, start, stop):
                for c in range(2):
                    nc.tensor.matmul(
                        psum,
                        i_sb,
                        p[:, c * 4 * W : (c + 1) * 4 * W],
                        start=(start and c == 0),
                        stop=(stop and c == 1),
                        skip_group_check=True,
                    )

            fold(psumA, tiles["b0p1"], True, False)
            fold(psumA, tiles["b0p3"], False, True)
            fold(psumB, tiles["b0p2"], True, True)

            # ---- DVE trees (in expected arrival order) ----
            def tree(p, dst=None):
                # [128,1024] -> [128,128]; result at p[:,0:W] or dst
                nc.vector.tensor_add(p[:, : 4 * W], p[:, : 4 * W], p[:, 4 * W :])
                nc.vector.tensor_add(p[:, : 2 * W], p[:, : 2 * W], p[:, 2 * W : 4 * W])
                if dst is None:
                    nc.vector.tensor_add(p[:, :W], p[:, :W], p[:, W : 2 * W])
                else:
                    nc.vector.tensor_add(dst, p[:, :W], p[:, W : 2 * W])

            tree(tiles["b1d2"])
            tree(b1d1)
            nc.gpsimd.tensor_add(
                tiles["b1d2"][:, 0:W], tiles["b1d2"][:, 0:W], b1d1[:, 0:W]
            )
            tree(tiles["b1d4"])
            tree(tiles["b1d3"])
            nc.gpsimd.tensor_add(
                tiles["b1d4"][:, 0:W], tiles["b1d4"][:, 0:W], tiles["b1d3"][:, 0:W]
            )
            nc.gpsimd.tensor_add(
                pads["xsp1"][:, 1 : W + 1],
                tiles["b1d2"][:, 0:W],
                tiles["b1d4"][:, 0:W],
            )
            tree(tiles["b0d"], dst=pads["xsp0t"][:, 1 : W + 1])
            with nc.allow_low_precision("bf16 partials; gate is 2e-2"):
                nc.vector.tensor_reduce(
                    pads["xsp0ra"][:, 1 : W + 1],
                    psumA[:, 0 : 4 * W].rearrange("p (j w) -> p w j", j=4),
                    axis=mybir.AxisListType.X,
                    op=mybir.AluOpType.add,
                )
                nc.vector.tensor_reduce(
                    pads["xsp0rb"][:, 1 : W + 1],
                    psumB[:, 0 : 4 * W].rearrange("p (j w) -> p w j", j=4),
                    axis=mybir.AxisListType.X,
                    op=mybir.AluOpType.add,
                )

            # ---- V matmuls ----
            def vmms(psum, xsp, start, stop):
                for k in range(3):
                    nc.tensor.matmul(
                        psum,
                        cm_sb[:, k * W : (k + 1) * W],
                        xsp[:, k : k + W],
                        start=(start and k == 0),
                        stop=(stop and k == 2),
                        skip_group_check=True,
                    )

            # b1 tail (hidden under b0 streaming)
            vmms(o_psum[1], pads["xsp1"], True, True)
            osb1 = spool.tile([H, W], f32, tag="osb1")
            nc.vector.tensor_scalar_add(osb1, o_psum[1], bias_sb)
            nc.sync.dma_start(out=out_ap[1, 0 : H // 2, :], in_=osb1[0 : H // 2, :])
            nc.scalar.dma_start(out=out_ap[1, H // 2 :, :], in_=osb1[H // 2 :, :])

            # b0 tail
            vmms(o_psum[0], pads["xsp0t"], True, False)
            vmms(o_psum[0], pads["xsp0ra"], False, False)
            vmms(o_psum[0], pads["xsp0rb"], False, True)
            osb0 = spool.tile([H, W], f32, tag="osb0")
            nc.vector.tensor_scalar_add(osb0, o_psum[0], bias_sb)
            nc.sync.dma_start(out=out_ap[0, 0 : H // 2, :], in_=osb0[0 : H // 2, :])
            nc.scalar.dma_start(out=out_ap[0, H // 2 :, :], in_=osb0[H // 2 :, :])

    nc.compile()
    return nc


def _get_program():
    global _PROGRAM
    if _PROGRAM is None:
        _PROGRAM = _build_program()
    return _PROGRAM


def _build_consts(K, bias, lambda_c, lambda_a):
    import ml_dtypes

    K = np.asarray(K, np.float32)
    lc = float(np.asarray(lambda_c))
    la = float(np.asarray(lambda_a))
    a = (lc + la) * K.sum(axis=0)  # column sums -> horizontal taps
    b = (la - lc) * K.sum(axis=1)  # row sums -> vertical taps
    eye = np.eye(H, dtype=np.float32)
    up = np.eye(H, k=1, dtype=np.float32)
    dn = np.eye(H, k=-1, dtype=np.float32)
    T = eye + up + dn
    Bm = b[1] * eye + b[2] * up + b[0] * dn
    vs = [np.ascontiguousarray((a[k] * T + Bm).T) for k in range(3)]
    cm = np.concatenate(vs + [eye], axis=1)
    cm16 = cm.astype(ml_dtypes.bfloat16)
    bias_col = np.full(
        (H, 1), np.asarray(bias, np.float32).reshape(-1)[0], np.float32
    )
    bias_bits = bias_col.view(np.uint16).view(ml_dtypes.bfloat16)  # [H, 2]
    return np.concatenate([cm16, bias_bits], axis=1)


def kernel(x, K, bias, lambda_c, lambda_a, _trace=False):
    global LAST_RESULTS
    import ml_dtypes
    from concourse.bass_utils import run_bass_kernel_spmd

    x = np.asarray(x, np.float32)
    cmb = _build_consts(K, bias, lambda_c, lambda_a)
    nc = _get_program()

    in_maps = []
    for core in range(N_CORES):
        shard = x[core * B_LOC : (core + 1) * B_LOC]  # [2, C, H, W]
        st = shard.transpose(2, 0, 1, 3).astype(ml_dtypes.bfloat16)  # [H,2,C,W]
        blocks = [
            cmb,
            st[:, 1, 0:8].reshape(H, PC),     # b1d1 (in head)
            st[:, 1, 16:24].reshape(H, PC),   # b1d3
            st[:, 0, 24:32].reshape(H, PC),   # b0d
            st[:, 0, 8:16].reshape(H, PC),    # b0p2
            st[:, 1, 8:16].reshape(H, PC),    # b1d2
            st[:, 1, 24:32].reshape(H, PC),   # b1d4
            st[:, 0, 0:8].reshape(H, PC),     # b0p1
            st[:, 0, 16:24].reshape(H, PC),   # b0p3
        ]
        xc = np.concatenate(blocks, axis=1)
        in_maps.append({"xc": np.ascontiguousarray(xc)})

    res = run_bass_kernel_spmd(
        nc, in_maps, core_ids=list(range(N_CORES)), trace=_trace
    )
    LAST_RESULTS = res
    out = np.concatenate([np.asarray(r["out"]) for r in res.results], axis=0)
    return out.reshape(B, 1, H, W).astype(np.float32, copy=False)


# revision 8
# speedup vs baseline: 1.1377x; 1.0412x over previous
"""CommutatorConv2d kernel for Trainium2 (Bass/Tile), 8-core data-parallel.

Math: the reference's commutator/anticommutator conv reduces exactly to a
single-channel 3x3 conv on the channel-summed input xs = x.sum(axis=1).
Writing the conv's horizontal taps as shifted copies and folding them into
the vertical band matrices gives a SINGLE matmul stage:

    out[b] = V0 @ shiftR(xs) + V1 @ xs + V2 @ shiftL(xs) + bias
    V_k = a[k]*T + Bm   (T tridiagonal-ones, Bm tridiagonal from K row
                         sums, a[k] from K column sums)

and since the V stage is linear, xs stays SPLIT into partial sums, each
in its own zero-edged pad buffer with its own 3 shifted V matmuls into
one accumulating PSUM group -- V matmuls pipeline at ~107ns on PE, so
extra V groups are far cheaper than merge adds on the DVE critical path.

v9 (bf16 in, f32 out): the HWDGE alternates WHOLE DMAs between the two
queues, so pieces arrive sequentially in global issue order at ~360GB/s.
Order: [head(cmat+b1d1), b1d2, b1p1, b1p2, b0d1, b0p1, b0d2, b0p2] --
batch 1 first (its tail hides under b0 streaming), then b0 alternating
DVE-tree / PE-fold pieces so both engines track the stream; the last
piece is a PE piece (fastest post-processing).  Per batch: 2 pieces DVE-
treed (d1 -> own pad, d2 -> own pad [b0] or gpsimd-merged [b1]), 2
pieces PE-identity-folded into PSUM, one tensor_reduce -> reduce pad.
"""

import numpy as np

B, C, H, W = 16, 32, 128, 128
N_CORES = 8
B_LOC = B // N_CORES

CMCOLS = 4 * W + 2
PC = 8 * W  # piece cols
N_JUNK = 8

_PROGRAM = None
LAST_RESULTS = None


def _build_program():
    import concourse.mybir as mybir
    from concourse import bacc
    from concourse.bass import MemorySpace
    from concourse.tile import TileContext

    bf16 = mybir.dt.bfloat16
    f32 = mybir.dt.float32
    nc = bacc.Bacc(
        "TRN2", target_bir_lowering=False, debug=False, num_devices=N_CORES
    )

    ncols = CMCOLS + 2 * C * W
    xc_dram = nc.dram_tensor("xc", (H, ncols), bf16, kind="ExternalInput")
    out_dram = nc.dram_tensor("out", (B_LOC, H, W), f32, kind="ExternalOutput")

    xc_ap = xc_dram.ap()
    out_ap = out_dram.ap()

    # global arrival order (queues alternate whole DMAs):
    #  1:head(cmat+b1d1)  2:b1d2  3:b1p1  4:b1p2  5:b0d1  6:b0p1  7:b0d2  8:b0p2
    # sync carries slots 1,3,5,7; scalar carries 2,4,6,8
    HEADC = CMCOLS + PC
    cof = {}
    c = HEADC
    for nm in ("b1p1", "b0d1", "b0d2", "b1d2", "b1p2", "b0p1", "b0p2"):
        cof[nm] = c
        c += PC

    with TileContext(nc) as tc:
        with (
            tc.tile_pool(name="xpool", bufs=1) as xpool,
            tc.tile_pool(name="spool", bufs=1) as spool,
            tc.tile_pool(name="psum", bufs=1, space=MemorySpace.PSUM) as ppool,
        ):
            scratch = spool.tile([H, 5 * W], bf16, tag="scratch")
            nc.gpsimd.memset(scratch, 0.0)
            pads = {}
            for nm in ("xsp1t", "xsp1r", "xsp0t1", "xsp0t2", "xsp0r"):
                t = spool.tile([H, W + 2], bf16, name=nm, tag=nm)
                nc.gpsimd.memset(t, 0.0)
                pads[nm] = t

            # ---- input DMAs ----
            head = xpool.tile([H, HEADC], bf16, tag="head")
            nc.sync.dma_start(out=head, in_=xc_ap[:, 0:HEADC])
            cm_sb = head[:, 0:CMCOLS]
            i_sb = cm_sb[:, 3 * W : 4 * W]
            bias_sb = cm_sb[:, 4 * W : 4 * W + 2].bitcast(f32)
            b1d1 = head[:, CMCOLS:HEADC]

            tiles = {}
            for nm in ("b1p1", "b0d1", "b0d2"):  # sync slots 3,5,7
                t = xpool.tile([H, PC], bf16, name=nm, tag=nm)
                nc.sync.dma_start(out=t, in_=xc_ap[:, cof[nm] : cof[nm] + PC])
                tiles[nm] = t
            for nm in ("b1d2", "b1p2", "b0p1", "b0p2"):  # scalar 2,4,6,8
                t = xpool.tile([H, PC], bf16, name=nm, tag=nm)
                nc.scalar.dma_start(out=t, in_=xc_ap[:, cof[nm] : cof[nm] + PC])
                tiles[nm] = t
            tiles["b1d1"] = b1d1

            # ---- PE warmup ----
            junk_psum = ppool.tile([H, 4 * W], f32, tag="junk")
            for _ in range(N_JUNK):
                nc.tensor.matmul(
                    junk_psum,
                    scratch[:, 0:W],
                    scratch[:, W : 5 * W],
                    start=True,
                    stop=True,
                    skip_group_check=True,
                )

            psum = {
                1: ppool.tile([H, 4 * W], f32, name="ps1", tag="ps1"),
                0: ppool.tile([H, 4 * W], f32, name="ps0", tag="ps0"),
            }
            o_psum = {
                1: ppool.tile([H, W], f32, name="op1", tag="op1"),
                0: ppool.tile([H, W], f32, name="op0", tag="op0"),
            }

            def fold(b, p, start, stop):
                for c in range(2):
                    nc.tensor.matmul(
                        psum[b],
                        i_sb,
                        p[:, c * 4 * W : (c + 1) * 4 * W],
                        start=(start and c == 0),
                        stop=(stop and c == 1),
                        skip_group_check=True,
                    )

            def tree(p, dst):
                # [128,1024] -> [128,128]; final add lands in dst pad
                nc.vector.tensor_add(p[:, : 4 * W], p[:, : 4 * W], p[:, 4 * W :])
                nc.vector.tensor_add(p[:, : 2 * W], p[:, : 2 * W], p[:, 2 * W : 4 * W])
                nc.vector.tensor_add(dst, p[:, :W], p[:, W : 2 * W])

            def reduce4(b, dst):
                with nc.allow_low_precision("bf16 partials; gate is 2e-2"):
                    nc.vector.tensor_reduce(
                        dst,
                        psum[b][:, 0 : 4 * W].rearrange("p (j w) -> p w j", j=4),
                        axis=mybir.AxisListType.X,
                        op=mybir.AluOpType.add,
                    )

            def vmms(b, xsp, start, stop):
                for k in range(3):
                    nc.tensor.matmul(
                        o_psum[b],
                        cm_sb[:, k * W : (k + 1) * W],
                        xsp[:, k : k + W],
                        start=(start and k == 0),
                        stop=(stop and k == 2),
                        skip_group_check=True,
                    )

            # ---- folds (PE order: b1p1, b1p2, b0p1, b0p2) ----
            fold(1, tiles["b1p1"], True, False)
            fold(1, tiles["b1p2"], False, True)
            # ---- b1 trees -> merged by gpsimd into xsp1t pad ----
            tree(tiles["b1d1"], tiles["b1d1"][:, 0:W])
            tree(tiles["b1d2"], tiles["b1d2"][:, 0:W])
            nc.gpsimd.tensor_add(
                pads["xsp1t"][:, 1 : W + 1],
                tiles["b1d1"][:, 0:W],
                tiles["b1d2"][:, 0:W],
            )
            fold(0, tiles["b0p1"], True, False)
            fold(0, tiles["b0p2"], False, True)
            # ---- DVE: b0d1 tree, b1 reduce, b0d2 tree, b0 reduce ----
            tree(tiles["b0d1"], pads["xsp0t1"][:, 1 : W + 1])
            reduce4(1, pads["xsp1r"][:, 1 : W + 1])
            tree(tiles["b0d2"], pads["xsp0t2"][:, 1 : W + 1])
            reduce4(0, pads["xsp0r"][:, 1 : W + 1])

            # ---- V matmuls + bias evac (ACT) + stores ----
            vmms(1, pads["xsp1t"], True, False)
            vmms(1, pads["xsp1r"], False, True)
            osb1 = spool.tile([H, W], f32, tag="osb1")
            nc.scalar.add(osb1, o_psum[1], add=bias_sb)
            nc.sync.dma_start(out=out_ap[1, 0 : H // 2, :], in_=osb1[0 : H // 2, :])
            nc.scalar.dma_start(out=out_ap[1, H // 2 :, :], in_=osb1[H // 2 :, :])

            vmms(0, pads["xsp0t1"], True, False)
            vmms(0, pads["xsp0t2"], False, False)
            vmms(0, pads["xsp0r"], False, True)
            osb0 = spool.tile([H, W], f32, tag="osb0")
            nc.scalar.add(osb0, o_psum[0], add=bias_sb)
            nc.sync.dma_start(out=out_ap[0, 0 : H // 2, :], in_=osb0[0 : H // 2, :])
            nc.scalar.dma_start(out=out_ap[0, H // 2 :, :], in_=osb0[H // 2 :, :])

    nc.compile()
    return nc


def _get_program():
    global _PROGRAM
    if _PROGRAM is None:
        _PROGRAM = _build_program()
    return _PROGRAM


def _build_consts(K, bias, lambda_c, lambda_a):
    import ml_dtypes

    K = np.asarray(K, np.float32)
    lc = float(np.asarray(lambda_c))
    la = float(np.asarray(lambda_a))
    a = (lc + la) * K.sum(axis=0)  # column sums -> horizontal taps
    b = (la - lc) * K.sum(axis=1)  # row sums -> vertical taps
    eye = np.eye(H, dtype=np.float32)
    up = np.eye(H, k=1, dtype=np.float32)
    dn = np.eye(H, k=-1, dtype=np.float32)
    T = eye + up + dn
    Bm = b[1] * eye + b[2] * up + b[0] * dn
    vs = [np.ascontiguousarray((a[k] * T + Bm).T) for k in range(3)]
    cm = np.concatenate(vs + [eye], axis=1)
    cm16 = cm.astype(ml_dtypes.bfloat16)
    bias_col = np.full(
        (H, 1), np.asarray(bias, np.float32).reshape(-1)[0], np.float32
    )
    bias_bits = bias_col.view(np.uint16).view(ml_dtypes.bfloat16)  # [H, 2]
    return np.concatenate([cm16, bias_bits], axis=1)


def kernel(x, K, bias, lambda_c, lambda_a, _trace=False):
    global LAST_RESULTS
    import ml_dtypes
    from concourse.bass_utils import run_bass_kernel_spmd

    x = np.asarray(x, np.float32)
    cmb = _build_consts(K, bias, lambda_c, lambda_a)
    nc = _get_program()

    in_maps = []
    for core in range(N_CORES):
        shard = x[core * B_LOC : (core + 1) * B_LOC]  # [2, C, H, W]
        st = shard.transpose(2, 0, 1, 3).astype(ml_dtypes.bfloat16)  # [H,2,C,W]
        blocks = [
            cmb,
            st[:, 1, 0:8].reshape(H, PC),     # b1d1 (in head)
            st[:, 1, 16:24].reshape(H, PC),   # b1p1
            st[:, 0, 0:8].reshape(H, PC),     # b0d1
            st[:, 0, 8:16].reshape(H, PC),    # b0d2
            st[:, 1, 8:16].reshape(H, PC),    # b1d2
            st[:, 1, 24:32].reshape(H, PC),   # b1p2
            st[:, 0, 16:24].reshape(H, PC),   # b0p1
            st[:, 0, 24:32].reshape(H, PC),   # b0p2
        ]
        xc = np.concatenate(blocks, axis=1)
        in_maps.append({"xc": np.ascontiguousarray(xc)})

    res = run_bass_kernel_spmd(
        nc, in_maps, core_ids=list(range(N_CORES)), trace=_trace
    )
    LAST_RESULTS = res
    out = np.concatenate([np.asarray(r["out"]) for r in res.results], axis=0)
    return out.reshape(B, 1, H, W).astype(np.float32, copy=False)


# revision 11
# speedup vs baseline: 1.1519x; 1.0125x over previous
"""CommutatorConv2d kernel for Trainium2 (Bass/Tile), 8-core data-parallel.

Math: the reference's commutator/anticommutator conv reduces exactly to a
single-channel 3x3 conv on the channel-summed input xs = x.sum(axis=1).
Writing the conv's horizontal taps as shifted copies and folding them into
the vertical band matrices gives a SINGLE matmul stage:

    out[b] = V0 @ shiftR(xs) + V1 @ xs + V2 @ shiftL(xs) + bias
    V_k = a[k]*T + Bm   (T tridiagonal-ones, Bm tridiagonal from K row
                         sums, a[k] from K column sums)

and since the V stage is linear, xs stays SPLIT into partial sums, each
in its own zero-edged pad buffer with its own 3 shifted V matmuls into
one accumulating PSUM group -- V matmuls pipeline at ~107ns on PE, so
extra V groups are far cheaper than merge adds on the DVE critical path.

v9 (bf16 in, f32 out): the HWDGE alternates WHOLE DMAs between the two
queues, so pieces arrive sequentially in global issue order at ~360GB/s.
Order: [head(cmat+b1d1), b1d2, b1p1, b1p2, b0d1, b0p1, b0d2, b0p2] --
batch 1 first (its tail hides under b0 streaming), then b0 alternating
DVE-tree / PE-fold pieces so both engines track the stream; the last
piece is a PE piece (fastest post-processing).  Per batch: 2 pieces DVE-
treed (d1 -> own pad, d2 -> own pad [b0] or gpsimd-merged [b1]), 2
pieces PE-identity-folded into PSUM, one tensor_reduce -> reduce pad.
"""

import numpy as np

B, C, H, W = 16, 32, 128, 128
N_CORES = 8
B_LOC = B // N_CORES

CMCOLS = 4 * W + 2
PC = 8 * W  # piece cols
N_JUNK = 8

_PROGRAM = None
LAST_RESULTS = None


def _build_program():
    import concourse.mybir as mybir
    from concourse import bacc
    from concourse.bass import MemorySpace
    from concourse.tile import TileContext

    bf16 = mybir.dt.bfloat16
    f32 = mybir.dt.float32
    nc = bacc.Bacc(
        "TRN2", target_bir_lowering=False, debug=False, num_devices=N_CORES
    )

    ncols = CMCOLS + 2 * C * W
    xc_dram = nc.dram_tensor("xc", (H, ncols), bf16, kind="ExternalInput")
    out_dram = nc.dram_tensor("out", (B_LOC, H, W), f32, kind="ExternalOutput")

    xc_ap = xc_dram.ap()
    out_ap = out_dram.ap()

    # global arrival order (queues alternate whole DMAs):
    #  1:head(cmat+b1d1)  2:b1d2  3:b1p1  4:b1p2  5:b0d1  6:b0p1  7:b0d2  8:b0p2
    # sync carries slots 1,3,5,7; scalar carries 2,4,6,8
    HEADC = CMCOLS + PC
    cof = {}
    c = HEADC
    for nm in ("b1p1", "b0d1", "b0d2", "b1d2", "b1p2", "b0p1", "b0p2"):
        cof[nm] = c
        c += PC

    with TileContext(nc) as tc:
        with (
            tc.tile_pool(name="xpool", bufs=1) as xpool,
            tc.tile_pool(name="spool", bufs=1) as spool,
            tc.tile_pool(name="psum", bufs=1, space=MemorySpace.PSUM) as ppool,
        ):
            scratch = spool.tile([H, 5 * W], bf16, tag="scratch")
            nc.gpsimd.memset(scratch, 0.0)
            pads = {}
            for nm in ("xsp1t", "xsp1r", "xsp0t1", "xsp0t2", "xsp0r"):
                t = spool.tile([H, W + 2], bf16, name=nm, tag=nm)
                nc.gpsimd.memset(t, 0.0)
                pads[nm] = t

            # ---- input DMAs ----
            head = xpool.tile([H, HEADC], bf16, tag="head")
            nc.sync.dma_start(out=head, in_=xc_ap[:, 0:HEADC])
            cm_sb = head[:, 0:CMCOLS]
            i_sb = cm_sb[:, 3 * W : 4 * W]
            bias_sb = cm_sb[:, 4 * W : 4 * W + 2].bitcast(f32)
            b1d1 = head[:, CMCOLS:HEADC]

            tiles = {}
            for nm in ("b1p1", "b0d1", "b0d2"):  # sync slots 3,5,7
                t = xpool.tile([H, PC], bf16, name=nm, tag=nm)
                nc.sync.dma_start(out=t, in_=xc_ap[:, cof[nm] : cof[nm] + PC])
                tiles[nm] = t
            for nm in ("b1d2", "b1p2", "b0p1", "b0p2"):  # scalar 2,4,6,8
                t = xpool.tile([H, PC], bf16, name=nm, tag=nm)
                nc.scalar.dma_start(out=t, in_=xc_ap[:, cof[nm] : cof[nm] + PC])
                tiles[nm] = t
            tiles["b1d1"] = b1d1

            # ---- PE warmup ----
            junk_psum = ppool.tile([H, 4 * W], f32, tag="junk")
            for _ in range(N_JUNK):
                nc.tensor.matmul(
                    junk_psum,
                    scratch[:, 0:W],
                    scratch[:, W : 5 * W],
                    start=True,
                    stop=True,
                    skip_group_check=True,
                )

            psum = {
                1: ppool.tile([H, 4 * W], f32, name="ps1", tag="ps1"),
                0: ppool.tile([H, 4 * W], f32, name="ps0", tag="ps0"),
            }
            o_psum = {
                1: ppool.tile([H, W], f32, name="op1", tag="op1"),
                0: ppool.tile([H, W], f32, name="op0", tag="op0"),
            }

            def fold(b, p, start, stop):
                for c in range(2):
                    nc.tensor.matmul(
                        psum[b],
                        i_sb,
                        p[:, c * 4 * W : (c + 1) * 4 * W],
                        start=(start and c == 0),
                        stop=(stop and c == 1),
                        skip_group_check=True,
                    )

            def tree(p, dst):
                # [128,1024] -> [128,128]; final add lands in dst pad
                nc.vector.tensor_add(p[:, : 4 * W], p[:, : 4 * W], p[:, 4 * W :])
                nc.vector.tensor_add(p[:, : 2 * W], p[:, : 2 * W], p[:, 2 * W : 4 * W])
                nc.vector.tensor_add(dst, p[:, :W], p[:, W : 2 * W])

            def reduce4(b, dst):
                # NCC_IBVF027: only ONE input may read PSUM, so a 2-add
                # chain is illegal; tensor_reduce does 4->1 in one op
                with nc.allow_low_precision("bf16 partials; gate is 2e-2"):
                    nc.vector.tensor_reduce(
                        dst,
                        psum[b][:, 0 : 4 * W].rearrange("p (j w) -> p w j", j=4),
                        axis=mybir.AxisListType.X,
                        op=mybir.AluOpType.add,
                    )

            def vmms(b, xsp, start, stop):
                for k in range(3):
                    nc.tensor.matmul(
                        o_psum[b],
                        cm_sb[:, k * W : (k + 1) * W],
                        xsp[:, k : k + W],
                        start=(start and k == 0),
                        stop=(stop and k == 2),
                        skip_group_check=True,
                    )

            # ---- b1: folds, trees, reduce, V, bias, store -- fully
            # emitted BEFORE b0's folds so b1's tail never queues behind
            # b0's straggler-blocked pieces on any engine ----
            fold(1, tiles["b1p1"], True, False)
            fold(1, tiles["b1p2"], False, True)
            tree(tiles["b1d1"], tiles["b1d1"][:, 0:W])
            tree(tiles["b1d2"], tiles["b1d2"][:, 0:W])
            nc.gpsimd.tensor_add(
                pads["xsp1t"][:, 1 : W + 1],
                tiles["b1d1"][:, 0:W],
                tiles["b1d2"][:, 0:W],
            )
            reduce4(1, pads["xsp1r"][:, 1 : W + 1])
            vmms(1, pads["xsp1t"], True, False)
            vmms(1, pads["xsp1r"], False, True)
            osb1 = spool.tile([H, W], f32, tag="osb1")
            nc.scalar.add(osb1, o_psum[1], add=bias_sb)
            nc.sync.dma_start(out=out_ap[1, 0 : H // 2, :], in_=osb1[0 : H // 2, :])
            nc.scalar.dma_start(out=out_ap[1, H // 2 :, :], in_=osb1[H // 2 :, :])

            # ---- b0 ----
            fold(0, tiles["b0p1"], True, False)
            fold(0, tiles["b0p2"], False, True)
            tree(tiles["b0d1"], pads["xsp0t1"][:, 1 : W + 1])
            tree(tiles["b0d2"], pads["xsp0t2"][:, 1 : W + 1])
            reduce4(0, pads["xsp0r"][:, 1 : W + 1])
            vmms(0, pads["xsp0t1"], True, False)
            vmms(0, pads["xsp0t2"], False, False)
            vmms(0, pads["xsp0r"], False, True)
            osb0 = spool.tile([H, W], f32, tag="osb0")
            nc.scalar.add(osb0, o_psum[0], add=bias_sb)
            nc.sync.dma_start(out=out_ap[0, 0 : H // 2, :], in_=osb0[0 : H // 2, :])
            nc.scalar.dma_start(out=out_ap[0, H // 2 :, :], in_=osb0[H // 2 :, :])

    nc.compile()
    return nc


def _get_program():
    global _PROGRAM
    if _PROGRAM is None:
        _PROGRAM = _build_program()
    return _PROGRAM


def _build_consts(K, bias, lambda_c, lambda_a):
    import ml_dtypes

    K = np.asarray(K, np.float32)
    lc = float(np.asarray(lambda_c))
    la = float(np.asarray(lambda_a))
    a = (lc + la) * K.sum(axis=0)  # column sums -> horizontal taps
    b = (la - lc) * K.sum(axis=1)  # row sums -> vertical taps
    eye = np.eye(H, dtype=np.float32)
    up = np.eye(H, k=1, dtype=np.float32)
    dn = np.eye(H, k=-1, dtype=np.float32)
    T = eye + up + dn
    Bm = b[1] * eye + b[2] * up + b[0] * dn
    vs = [np.ascontiguousarray((a[k] * T + Bm).T) for k in range(3)]
    cm = np.concatenate(vs + [eye], axis=1)
    cm16 = cm.astype(ml_dtypes.bfloat16)
    bias_col = np.full(
        (H, 1), np.asarray(bias, np.float32).reshape(-1)[0], np.float32
    )
    bias_bits = bias_col.view(np.uint16).view(ml_dtypes.bfloat16)  # [H, 2]
    return np.concatenate([cm16, bias_bits], axis=1)


def kernel(x, K, bias, lambda_c, lambda_a, _trace=False):
    global LAST_RESULTS
    import ml_dtypes
    from concourse.bass_utils import run_bass_kernel_spmd

    x = np.asarray(x, np.float32)
    cmb = _build_consts(K, bias, lambda_c, lambda_a)
    nc = _get_program()

    in_maps = []
    for core in range(N_CORES):
        shard = x[core * B_LOC : (core + 1) * B_LOC]  # [2, C, H, W]
        st = shard.transpose(2, 0, 1, 3).astype(ml_dtypes.bfloat16)  # [H,2,C,W]
        blocks = [
            cmb,
            st[:, 1, 0:8].reshape(H, PC),     # b1d1 (in head)
            st[:, 1, 16:24].reshape(H, PC),   # b1p1
            st[:, 0, 0:8].reshape(H, PC),     # b0d1
            st[:, 0, 8:16].reshape(H, PC),    # b0d2
            st[:, 1, 8:16].reshape(H, PC),    # b1d2
            st[:, 1, 24:32].reshape(H, PC),   # b1p2
            st[:, 0, 16:24].reshape(H, PC),   # b0p1
            st[:, 0, 24:32].reshape(H, PC),   # b0p2
        ]
        xc = np.concatenate(blocks, axis=1)
        in_maps.append({"xc": np.ascontiguousarray(xc)})

    res = run_bass_kernel_spmd(
        nc, in_maps, core_ids=list(range(N_CORES)), trace=_trace
    )
    LAST_RESULTS = res
    out = np.concatenate([np.asarray(r["out"]) for r in res.results], axis=0)
    return out.reshape(B, 1, H, W).astype(np.float32, copy=False)
